# revision 1
# baseline (speedup 1.0000x reference)
"""Trainium2 Bass kernel for nn_NNSDecoder (gnn_message_passing).

Reference computation (B=16, N=501, D=128, H=4):
    out[b,i,j] = fc3 . relu(fc2^T relu(feat @ fc1 + b1) + b2) + b3
    feat[b,i,j] = [cp_pre[b,i], cp_post[b,i], cd_pre[b,j], cd_post[b,j]]  (4H=16)

Key algebra: compat[b,n,h] = x[b,n] . (Wk[h] Wq[h]^T q_b), so every
pickup/delivery-side term is linear in h_hat / h_nb rows.  Folding the
head projections and fc1 together gives per-batch 128x32 maps:
    A[b] = h_hat[b] @ G_A1 + h_nb[b] @ G_A2          (N x 32, row/i term)
    C[b] = h_hat[b] @ G_C1 + h_nb[b] @ G_C2          (N x 32, col/j term)
    out[b,i,j] = w3 . relu(W2^T relu(A[b,i] + C[b,j] + b1) + b2) + b3

Device pipeline per batch (i-tiles of 4 rows; pairs of i-tiles):
  - prep: A^T (32 x NP) and C^T stacked 4x (128 x NP) in PSUM;
    crep = C^T + b1 (bf16), a4 = i-tile column layout of A (f32).
  - per pair (t0,t1): X(t) = relu(crep + a4[:,t]) on DVE (bf16 SBUF,
    4x perf mode), fc2 block-diag bf16 matmuls into the two halves of a
    2-bank PSUM tile, one paired Y = relu(pz2 + b2) (ScalarE/DVE split),
    fc3 column-position-packed matmuls into po2 (2 banks = 8 tiles).
  - per po2 (32 out rows): one ScalarE copy to SBUF, one 32-row DMA
    (output DRAM padded to 512 rows so no edge cases).

Sharding: batch dim 16 -> 8 cores x 2 batches (data parallel, weights
replicated). Full inputs in, full output out.
"""

import numpy as np

B, N, D, H = 16, 501, 128, 4
NCORES = 8
BPC = B // NCORES  # batches per core
NP = 504  # padded N: multiple of 8, fits one PSUM bank (<=512 f32)
NT = NP // 4  # 126 i-tiles of 4 rows each
PB = 512  # PSUM bank width in f32; pair tiles use bank-aligned halves
NPAIR = NT // 2  # 63 pairs of i-tiles
NOUT = 512  # padded output rows (uniform 32-row DMAs)

# Y-pair engine split: pairs with (p % 8) < Y_DVE_MOD go to DVE, rest ScalarE
Y_DVE_MOD = 3

_cache = {}


def _build_program():
    import concourse.bacc as bacc
    import concourse.mybir as mybir
    from concourse.tile import TileContext
    from concourse.bass_types import AP

    F32 = mybir.dt.float32
    BF16 = mybir.dt.bfloat16
    nc = bacc.Bacc("TRN2", target_bir_lowering=False, debug=False, num_devices=1)

    hhT = nc.dram_tensor("hhT", [BPC, D, NP], BF16, kind="ExternalInput")
    hnT = nc.dram_tensor("hnT", [BPC, D, NP], BF16, kind="ExternalInput")
    g1a = nc.dram_tensor("g1a", [BPC, D, 32], BF16, kind="ExternalInput")
    g2a = nc.dram_tensor("g2a", [BPC, D, 32], BF16, kind="ExternalInput")
    g1c = nc.dram_tensor("g1c", [BPC, D, 128], BF16, kind="ExternalInput")
    g2c = nc.dram_tensor("g2c", [BPC, D, 128], BF16, kind="ExternalInput")
    w2d = nc.dram_tensor("w2d", [D, 128], BF16, kind="ExternalInput")
    w3d = nc.dram_tensor("w3d", [D, 4], BF16, kind="ExternalInput")
    b1r = nc.dram_tensor("b1r", [D, 1], F32, kind="ExternalInput")
    b2r = nc.dram_tensor("b2r", [D, 1], F32, kind="ExternalInput")
    out = nc.dram_tensor("out", [BPC, NOUT, N], F32, kind="ExternalOutput")

    add = mybir.AluOpType.add
    amax = mybir.AluOpType.max
    Relu = mybir.ActivationFunctionType.Relu
    Identity = mybir.ActivationFunctionType.Identity

    with TileContext(nc) as tc:
        with (
            tc.tile_pool(name="const", bufs=1) as cpool,
            tc.tile_pool(name="batch", bufs=2) as bpool,
            tc.tile_pool(name="x", bufs=12) as xpool,
            tc.tile_pool(name="y", bufs=6) as ypool,
            tc.tile_pool(name="o", bufs=2) as opool,
            tc.tile_pool(name="pz", bufs=3, space="PSUM") as pzpool,
            tc.tile_pool(name="po", bufs=1, space="PSUM") as popool,
        ):
            w2t = cpool.tile([D, 128], BF16)
            nc.sync.dma_start(w2t[:], w2d.ap()[:, :])
            w3t = cpool.tile([D, 4], BF16)
            nc.sync.dma_start(w3t[:], w3d.ap()[:, :])
            b1t = cpool.tile([D, 1], F32)
            nc.sync.dma_start(b1t[:], b1r.ap()[:, :])
            b2t = cpool.tile([D, 1], F32)
            nc.sync.dma_start(b2t[:], b2r.ap()[:, :])

            # prefetch all batches' inputs up front (bpool bufs=2 holds both)
            batch_in = []
            for b in range(BPC):
                e1 = nc.sync
                e2 = nc.scalar if b == 0 else nc.sync
                hh = bpool.tile([D, NP], BF16, tag="hh", name=f"hh{b}")
                e1.dma_start(hh[:], hhT.ap()[b, :, :])
                hn = bpool.tile([D, NP], BF16, tag="hn", name=f"hn{b}")
                e2.dma_start(hn[:], hnT.ap()[b, :, :])
                g1ct = bpool.tile([D, 128], BF16, tag="g1c", name=f"g1ct{b}")
                e1.dma_start(g1ct[:], g1c.ap()[b, :, :])
                g2ct = bpool.tile([D, 128], BF16, tag="g2c", name=f"g2ct{b}")
                e2.dma_start(g2ct[:], g2c.ap()[b, :, :])
                g1at = bpool.tile([D, 32], BF16, tag="g1a", name=f"g1at{b}")
                e1.dma_start(g1at[:], g1a.ap()[b, :, :])
                g2at = bpool.tile([D, 32], BF16, tag="g2a", name=f"g2at{b}")
                e2.dma_start(g2at[:], g2a.ap()[b, :, :])
                batch_in.append((hh, hn, g1at, g2at, g1ct, g2ct))

            # prep BOTH batches up front: crep/a4 live in SBUF and the
            # transient pprep PSUM slot frees as soon as they are built,
            # so batch 1 starts with zero boundary bubble.
            batch_prep = []
            for b in range(BPC):
                hh, hn, g1at, g2at, g1ct, g2ct = batch_in[b]
                pprep = pzpool.tile([D, 2 * PB], F32, tag="pz", name=f"pprep{b}")
                pac = pprep[:, 0:NP]
                paa = pprep[0:32, PB : PB + NP]
                nc.tensor.matmul(paa, g1at[:], hh[:], start=True, stop=False)
                nc.tensor.matmul(paa, g2at[:], hn[:], start=False, stop=True)
                nc.tensor.matmul(pac, g1ct[:], hh[:], start=True, stop=False)
                nc.tensor.matmul(pac, g2ct[:], hn[:], start=False, stop=True)
                crep = bpool.tile([D, NP], BF16, tag="crep", name=f"crep{b}")
                nc.scalar.activation(crep[:], pac, Identity, bias=b1t[:, 0:1])
                a4 = bpool.tile([D, NT], F32, tag="a4", name=f"a4_{b}")
                paa_r = paa.rearrange("p (t r) -> p r t", r=4)
                for r in range(4):
                    nc.vector.tensor_copy(
                        a4[32 * r : 32 * r + 32, :], paa_r[:, r, :]
                    )
                batch_prep.append((crep, a4))

            for b in range(BPC):
                crep, a4 = batch_prep[b]

                po2 = None
                xq = {}

                def emit_x(p):
                    xs = []
                    for t in (2 * p, 2 * p + 1):
                        x = xpool.tile([D, NP], BF16, name=f"x{b}_{t}", tag="x")
                        nc.vector.tensor_scalar(
                            out=x[:],
                            in0=crep[:],
                            scalar1=a4[:, t : t + 1],
                            scalar2=0.0,
                            op0=add,
                            op1=amax,
                        )
                        xs.append(x)
                    xq[p] = xs

                po2_box = [None]

                def do_fc3(p, y2):
                    po2 = po2_box[0]
                    for s, t in enumerate((2 * p, 2 * p + 1)):
                        u = t % 4
                        q = (t // 4) % 2
                        if t % 8 == 0:
                            # tail groups borrow now-idle pz slots so the
                            # final copies/DMAs pipeline instead of
                            # serializing on the single po buffer
                            tail = b == BPC - 1 and t >= NT - 24
                            pool = pzpool if tail else popool
                            tag = "pz" if tail else "po2"
                            po2 = pool.tile(
                                [D, 2 * PB], F32, name=f"po2_{b}_{t}", tag=tag
                            )
                            po2_box[0] = po2
                        nc.tensor.matmul(
                            po2[32 * u : 32 * u + 4, q * PB : q * PB + NP],
                            w3t[:],
                            y2[:, s * PB : s * PB + NP],
                            start=True,
                            stop=True,
                            tile_position=(0, 32 * u),
                        )
                        if t == NT - 1 or t % 8 == 7:
                            # po2 complete (or end of batch): copy + DMA out
                            ob = opool.tile([D, 2 * PB], F32)
                            nc.scalar.copy(ob[:], po2[:])
                            gi = t // 8  # po2 index; rows 32*gi .. 32*gi+31
                            base = ob[:, :]
                            pitch = base.ap[0][0]  # partition pitch (elements)
                            seg = out.ap()[
                                b, 32 * gi : 32 * gi + 32, :
                            ].rearrange("(q u r) n -> r u q n", q=2, u=4)
                            for r in range(4):
                                src = AP(
                                    base.tensor,
                                    base.offset + r * pitch,
                                    [
                                        [32 * pitch, 4],  # u: partition group
                                        [PB, 2],  # q: column half
                                        [1, N],  # j
                                    ],
                                )
                                nc.sync.dma_start(seg[r], src)

                emit_x(0)
                pend = []
                for p in range(NPAIR):
                    if p + 1 < NPAIR:
                        emit_x(p + 1)
                    # fc2 for the two tiles of this pair
                    pz2 = pzpool.tile([D, 2 * PB], F32, tag="pz")
                    for s, x in enumerate(xq.pop(p)):
                        nc.tensor.matmul(
                            pz2[:, s * PB : s * PB + NP],
                            w2t[:],
                            x[:],
                            start=True,
                            stop=True,
                        )
                    # paired Y = relu(pz2 + b2) -> bf16
                    y2 = ypool.tile([D, 2 * PB], BF16, tag="y2")
                    y2v = y2.rearrange("p (q n) -> p q n", q=2)[:, :, 0:NP]
                    pz2v = pz2.rearrange("p (q n) -> p q n", q=2)[:, :, 0:NP]
                    if p % 13 in (0, 3, 6, 10):
                        nc.vector.tensor_scalar(
                            out=y2v,
                            in0=pz2v,
                            scalar1=b2t[:, 0:1],
                            scalar2=0.0,
                            op0=add,
                            op1=amax,
                        )
                    else:
                        nc.scalar.activation(
                            y2v, pz2v, Relu, bias=b2t[:, 0:1]
                        )
                    # fc3 trails two pairs behind so it never blocks fc2 on PE
                    pend.append((p, y2))
                    if len(pend) > 2:
                        do_fc3(*pend.pop(0))
                for item in pend:
                    do_fc3(*item)

    nc.compile()
    return nc


def _host_prep(h_hat, pos_pickup, pos_delivery, solution, Wq1, Wk1, Wq2, Wk2, fc1_w):
    """Per-batch tiny maps G (128x32 each) + transposed/padded node features."""
    import ml_dtypes

    f32 = np.float32
    bf16 = ml_dtypes.bfloat16
    h_hat = np.asarray(h_hat, f32)
    pp = np.asarray(pos_pickup).astype(np.int64)
    pd = np.asarray(pos_delivery).astype(np.int64)
    sol = np.asarray(solution).astype(np.int64)
    Wq1 = np.asarray(Wq1, f32)
    Wk1 = np.asarray(Wk1, f32)
    Wq2 = np.asarray(Wq2, f32)
    Wk2 = np.asarray(Wk2, f32)
    fc1_w = np.asarray(fc1_w, f32)

    hhT = np.zeros((B, D, NP), bf16)
    hnT = np.zeros((B, D, NP), bf16)
    g1a = np.zeros((B, D, 32), bf16)
    g2a = np.zeros((B, D, 32), bf16)
    g1c = np.zeros((B, D, 128), bf16)
    g2c = np.zeros((B, D, 128), bf16)

    for b in range(B):
        hb = h_hat[b]  # (N, D)
        hnb = hb[sol[b]]  # (N, D) gathered neighbours
        hhT[b, :, :N] = hb.T
        hnT[b, :, :N] = hnb.T
        p = hb[pp[b]]  # (D,)
        dv = hb[pd[b]]
        # u[h] = Wk[h] @ (Wq[h]^T @ q): compat[n,h] = x[n] . u[h]
        U1p = np.stack([Wk1[h] @ (Wq1[h].T @ p) for h in range(H)], axis=1)
        U2p = np.stack([Wk2[h] @ (Wq2[h].T @ p) for h in range(H)], axis=1)
        U1d = np.stack([Wk1[h] @ (Wq1[h].T @ dv) for h in range(H)], axis=1)
        U2d = np.stack([Wk2[h] @ (Wq2[h].T @ dv) for h in range(H)], axis=1)
        g1a[b] = U1p @ fc1_w[0:4]  # h_hat -> A
        g2a[b] = U2p @ fc1_w[4:8]  # h_nb  -> A
        gc1 = U1d @ fc1_w[8:12]  # h_hat -> C
        gc2 = U2d @ fc1_w[12:16]  # h_nb  -> C
        g1c[b] = np.tile(gc1, (1, 4))
        g2c[b] = np.tile(gc2, (1, 4))
    return hhT, hnT, g1a, g2a, g1c, g2c


_last_results = None


def kernel(
    h_hat,
    pos_pickup,
    pos_delivery,
    solution,
    Wq1,
    Wk1,
    Wq2,
    Wk2,
    fc1_w,
    fc1_b,
    fc2_w,
    fc2_b,
    fc3_w,
    fc3_b,
):
    global _last_results
    import ml_dtypes
    from concourse.bass_utils import run_bass_kernel_spmd

    f32 = np.float32
    bf16 = ml_dtypes.bfloat16
    fc2_w = np.asarray(fc2_w, f32)
    fc1_b = np.asarray(fc1_b, f32)
    fc2_b = np.asarray(fc2_b, f32)
    fc3_w = np.asarray(fc3_w, f32)
    fc3_b = np.asarray(fc3_b, f32)

    hhT, hnT, g1a, g2a, g1c, g2c = _host_prep(
        h_hat, pos_pickup, pos_delivery, solution, Wq1, Wk1, Wq2, Wk2,
        np.asarray(fc1_w, f32),
    )

    # block-diagonal packed MLP weights (4 independent 32-blocks)
    w2d = np.zeros((D, 128), f32)
    w3d = np.zeros((D, 4), f32)
    for r in range(4):
        w2d[32 * r : 32 * r + 32, 32 * r : 32 * r + 32] = fc2_w
        w3d[32 * r : 32 * r + 32, r : r + 1] = fc3_w.reshape(32, 1)
    b1r = np.tile(fc1_b.reshape(32, 1), (4, 1)).astype(f32)
    b2r = np.tile(fc2_b.reshape(32, 1), (4, 1)).astype(f32)

    if "nc" not in _cache:
        _cache["nc"] = _build_program()
    nc = _cache["nc"]

    in_maps = []
    for c in range(NCORES):
        bs = slice(BPC * c, BPC * (c + 1))
        in_maps.append(
            {
                "hhT": np.ascontiguousarray(hhT[bs]),
                "hnT": np.ascontiguousarray(hnT[bs]),
                "g1a": np.ascontiguousarray(g1a[bs]),
                "g2a": np.ascontiguousarray(g2a[bs]),
                "g1c": np.ascontiguousarray(g1c[bs]),
                "g2c": np.ascontiguousarray(g2c[bs]),
                "w2d": w2d.astype(bf16),
                "w3d": w3d.astype(bf16),
                "b1r": b1r,
                "b2r": b2r,
            }
        )

    res = run_bass_kernel_spmd(nc, in_maps, core_ids=list(range(NCORES)))
    _last_results = res

    out = np.concatenate(
        [res.results[c]["out"][:, :N, :] for c in range(NCORES)], axis=0
    )
    b3 = float(fc3_b.reshape(-1)[0])
    if b3 != 0.0:
        out = out + b3
    return out.astype(f32)



# revision 14
# speedup vs baseline: 1.1267x; 1.1267x over previous
"""Trainium2 Bass kernel for nn_NNSDecoder (gnn_message_passing).

Reference computation (B=16, N=501, D=128, H=4):
    out[b,i,j] = fc3 . relu(fc2^T relu(feat @ fc1 + b1) + b2) + b3
    feat[b,i,j] = [cp_pre[b,i], cp_post[b,i], cd_pre[b,j], cd_post[b,j]]

Every compatibility term is linear in h_hat / h_nb rows, so folding the
head projections and fc1 gives per-batch N x 32 maps computed ON HOST
(O(N) work):
    A[b] = h_hat[b] @ G_A1 + h_nb[b] @ G_A2      (row/i term)
    C[b] = h_hat[b] @ G_C1 + h_nb[b] @ G_C2      (col/j term)
    out[b,i,j] = w3 . relu(W2^T relu(A[b,i] + C[b,j] + b1) + b2) + b3

Device inputs per batch: crep (bf16 [128, 504] = C^T + b1 stacked 4x)
and a4 (f32 [128, 126] = A in column-per-4-row-tile layout).

Device pipeline per batch (i-tiles of 4 rows, processed in pairs):
  - X(t) = relu(crep + a4[:,t])  on DVE (tensor_scalar, 2x mode)
  - fc2: ONE 1008-wide block-diag bf16 matmul per pair -> 2-bank pz2
  - Y = relu(pz2 + b2) -> bf16, one 1008-wide op (Scalar/DVE split)
  - fc3: ONE 1008-wide matmul per pair with one of 8 row-offset weight
    variants, accumulating 32 pairs (256 output rows, two 504-col
    halves per partition) into a dense 2-bank po2; po2 is copied once
    per 32 pairs and DMA'd out with an affine row remap.

Sharding: batch dim 16 -> 8 cores x 2 batches (data parallel).
"""

import numpy as np

B, N, D, H = 16, 501, 128, 4
NCORES = 8
BPC = B // NCORES  # batches per core
NP = 504  # padded N
NT = NP // 4  # 126 i-tiles of 4 rows
NPAIR = NT // 2  # 63 pairs per batch
PB = 512
NOUT = 512  # padded output rows per batch
PPPO = 32  # pairs per dense po2 (256 rows)

# Y-pair engine split: of every 6 pairs, which go to Scalar vs DVE
Y_SPLIT = ("S", "S", "S", "S", "S", "D")

_cache = {}


def _build_program():
    import concourse.bacc as bacc
    import concourse.mybir as mybir
    from concourse.tile import TileContext
    from concourse.bass_types import AP

    F32 = mybir.dt.float32
    BF16 = mybir.dt.bfloat16
    nc = bacc.Bacc("TRN2", target_bir_lowering=False, debug=False, num_devices=1)

    crep_d = nc.dram_tensor("crep", [BPC, D, NP], BF16, kind="ExternalInput")
    a4_d = nc.dram_tensor("a4", [BPC, D, NT], F32, kind="ExternalInput")
    w2_d = nc.dram_tensor("w2d", [D, D], BF16, kind="ExternalInput")
    w3_d = nc.dram_tensor("w3d", [D, 8 * 32], BF16, kind="ExternalInput")
    b2_d = nc.dram_tensor("b2r", [D, 1], F32, kind="ExternalInput")
    out = nc.dram_tensor("out", [BPC, NOUT, N], F32, kind="ExternalOutput")

    add = mybir.AluOpType.add
    amax = mybir.AluOpType.max
    Relu = mybir.ActivationFunctionType.Relu

    with TileContext(nc) as tc:
        with (
            tc.tile_pool(name="const", bufs=1) as cpool,
            tc.tile_pool(name="batch", bufs=2) as bpool,
            tc.tile_pool(name="x", bufs=12) as xpool,
            tc.tile_pool(name="y", bufs=6) as ypool,
            tc.tile_pool(name="o", bufs=2) as opool,
            tc.tile_pool(name="pz", bufs=3, space="PSUM") as pzpool,
            tc.tile_pool(name="po", bufs=1, space="PSUM") as popool,
        ):
            # ---- constants ----
            w2t = cpool.tile([D, D], BF16)
            nc.sync.dma_start(w2t[:], w2_d.ap()[:, :])
            w3t = cpool.tile([D, 8 * 32], BF16)
            nc.scalar.dma_start(w3t[:], w3_d.ap()[:, :])
            b2t = cpool.tile([D, 1], F32)
            nc.gpsimd.dma_start(b2t[:], b2_d.ap()[:, :])

            # ---- per-batch inputs: crep split over 4 engines' queues ----
            batch_in = []
            for b in range(BPC):
                crep = bpool.tile([D, NP], BF16, tag="crep", name=f"crep{b}")
                a4 = bpool.tile([D, NT], F32, tag="a4", name=f"a4_{b}")
                engs = [nc.sync, nc.scalar, nc.gpsimd, nc.sync]
                for c, e in enumerate(engs):
                    e.dma_start(
                        crep[32 * c : 32 * (c + 1), :],
                        crep_d.ap()[b, 32 * c : 32 * (c + 1), :],
                    )
                (nc.scalar if b == 0 else nc.gpsimd).dma_start(
                    a4[:], a4_d.ap()[b, :, :]
                )
                batch_in.append((crep, a4))

            for b in range(BPC):
                crep, a4 = batch_in[b]

                xq = {}

                def emit_x(t):
                    # pair tile [D, 1008]: halves for tiles 2p / 2p+1
                    p_, h = divmod(t, 2)
                    if h == 0:
                        xq[p_] = xpool.tile(
                            [D, 2 * NP], BF16, name=f"x{b}_{p_}", tag="x"
                        )
                    x = xq[p_]
                    nc.vector.tensor_scalar(
                        out=x[:, h * NP : h * NP + NP],
                        in0=crep[:],
                        scalar1=a4[:, t : t + 1],
                        scalar2=0.0,
                        op0=add,
                        op1=amax,
                    )

                po2_box = [None]

                def flush_po2(po2, pidx):
                    # dense po2: partition P = 4*sp + g, col 512q + j
                    #   -> out row 256*pidx + 8*sp + 4q + g
                    ob = opool.tile([D, 2 * PB], F32)
                    nc.scalar.copy(ob[:], po2[:])
                    base = ob[:, :]
                    pitch = base.ap[0][0]
                    for g in range(4):
                        for q in range(2):
                            src = AP(
                                base.tensor,
                                base.offset + g * pitch + q * PB,
                                [
                                    [4 * pitch, PPPO],  # sp
                                    [1, N],  # j
                                ],
                            )
                            seg = out.ap()[b, :, :]
                            dst = AP(
                                seg.tensor,
                                seg.offset + (256 * pidx + 4 * q + g) * N,
                                [
                                    [8 * N, PPPO],  # sp
                                    [1, N],  # j
                                ],
                            )
                            nc.sync.dma_start(dst, src)

                def do_fc3(p, y2):
                    sp = p % PPPO
                    cg, u = divmod(sp, 8)
                    if sp == 0:
                        po2_box[0] = popool.tile(
                            [D, 2 * PB], F32, name=f"po2_{b}_{p // PPPO}", tag="po"
                        )
                    po2 = po2_box[0]
                    for q in (0, 1):
                        nc.tensor.matmul(
                            po2[32 * cg : 32 * cg + 32, q * PB : q * PB + NP],
                            w3t[:, 32 * u : 32 * u + 32],
                            y2[:, q * PB : q * PB + NP],
                            start=(u == 0),
                            stop=(u == 7 or p == NPAIR - 1),
                            tile_position=(0, 32 * cg),
                            skip_group_check=True,
                        )
                    if sp == PPPO - 1 or p == NPAIR - 1:
                        flush_po2(po2, p // PPPO)

                for t in range(4):
                    emit_x(t)
                pend = []
                for p in range(NPAIR):
                    for t in (2 * p + 4, 2 * p + 5):
                        if t < NT:
                            emit_x(t)
                    # fc2: two 504-wide matmuls into bank-aligned pz2 halves
                    pz2 = pzpool.tile([D, 2 * PB], F32, tag="pz")
                    xpair = xq.pop(p)
                    for q in (0, 1):
                        nc.tensor.matmul(
                            pz2[:, q * PB : q * PB + NP],
                            w2t[:],
                            xpair[:, q * NP : q * NP + NP],
                            start=True,
                            stop=True,
                        )
                    # Y = relu(pz2 + b2) -> bf16, one 1008-elem strided op
                    y2 = ypool.tile([D, 2 * PB], BF16, tag="y2")
                    y2v = y2.rearrange("p (q n) -> p q n", q=2)[:, :, 0:NP]
                    pz2v = pz2.rearrange("p (q n) -> p q n", q=2)[:, :, 0:NP]
                    if Y_SPLIT[p % len(Y_SPLIT)] == "D":
                        nc.vector.tensor_scalar(
                            out=y2v,
                            in0=pz2v,
                            scalar1=b2t[:, 0:1],
                            scalar2=0.0,
                            op0=add,
                            op1=amax,
                        )
                    else:
                        nc.scalar.activation(y2v, pz2v, Relu, bias=b2t[:, 0:1])
                    pend.append((p, y2))
                    if len(pend) > 2:
                        do_fc3(*pend.pop(0))
                for item in pend:
                    do_fc3(*item)

    nc.compile()
    return nc


def _host_prep(h_hat, pos_pickup, pos_delivery, solution, Wq1, Wk1, Wq2, Wk2,
               fc1_w, fc1_b):
    """Host: per-batch A, C maps (O(N*D) work), then crep/a4 layouts."""
    import ml_dtypes

    f32 = np.float32
    bf16 = ml_dtypes.bfloat16
    h_hat = np.asarray(h_hat, f32)
    pp = np.asarray(pos_pickup).astype(np.int64)
    pd = np.asarray(pos_delivery).astype(np.int64)
    sol = np.asarray(solution).astype(np.int64)
    Wq1 = np.asarray(Wq1, f32)
    Wk1 = np.asarray(Wk1, f32)
    Wq2 = np.asarray(Wq2, f32)
    Wk2 = np.asarray(Wk2, f32)
    fc1_w = np.asarray(fc1_w, f32)
    fc1_b = np.asarray(fc1_b, f32)

    crep = np.zeros((B, D, NP), bf16)
    a4 = np.zeros((B, D, NT), f32)

    for b in range(B):
        hb = h_hat[b]
        hnb = hb[sol[b]]
        p = hb[pp[b]]
        dv = hb[pd[b]]
        U1p = np.stack([Wk1[h] @ (Wq1[h].T @ p) for h in range(H)], axis=1)
        U2p = np.stack([Wk2[h] @ (Wq2[h].T @ p) for h in range(H)], axis=1)
        U1d = np.stack([Wk1[h] @ (Wq1[h].T @ dv) for h in range(H)], axis=1)
        U2d = np.stack([Wk2[h] @ (Wq2[h].T @ dv) for h in range(H)], axis=1)
        g1a = (U1p @ fc1_w[0:4]).astype(bf16).astype(f32)
        g2a = (U2p @ fc1_w[4:8]).astype(bf16).astype(f32)
        g1c = (U1d @ fc1_w[8:12]).astype(bf16).astype(f32)
        g2c = (U2d @ fc1_w[12:16]).astype(bf16).astype(f32)
        hbq = hb.astype(bf16).astype(f32)
        hnq = hnb.astype(bf16).astype(f32)
        A = hbq @ g1a + hnq @ g2a  # (N, 32)
        C = hbq @ g1c + hnq @ g2c  # (N, 32)
        CB = (C + fc1_b).astype(bf16)  # (N, 32)
        ct = np.tile(CB.T, (4, 1))  # (128, N)
        crep[b, :, :N] = ct
        crep[b, :, N:] = np.tile(fc1_b.reshape(32, 1).astype(bf16), (4, 1))
        Ap = np.zeros((NP, 32), f32)
        Ap[:N] = A
        a4[b] = Ap.reshape(NT, 4, 32).transpose(1, 2, 0).reshape(D, NT)
    return crep, a4


_last_results = None


def kernel(
    h_hat,
    pos_pickup,
    pos_delivery,
    solution,
    Wq1,
    Wk1,
    Wq2,
    Wk2,
    fc1_w,
    fc1_b,
    fc2_w,
    fc2_b,
    fc3_w,
    fc3_b,
):
    global _last_results
    import ml_dtypes
    from concourse.bass_utils import run_bass_kernel_spmd

    f32 = np.float32
    bf16 = ml_dtypes.bfloat16
    fc2_w = np.asarray(fc2_w, f32)
    fc2_b = np.asarray(fc2_b, f32)
    fc3_w = np.asarray(fc3_w, f32)
    fc3_b = np.asarray(fc3_b, f32)

    crep, a4 = _host_prep(
        h_hat, pos_pickup, pos_delivery, solution, Wq1, Wk1, Wq2, Wk2,
        np.asarray(fc1_w, f32), np.asarray(fc1_b, f32),
    )

    # block-diagonal packed fc2; 8 row-offset variants of fc3
    w2d = np.zeros((D, D), f32)
    for r in range(4):
        w2d[32 * r : 32 * r + 32, 32 * r : 32 * r + 32] = fc2_w
    w3d = np.zeros((D, 8, 32), f32)
    for u in range(8):
        for g in range(4):
            w3d[32 * g : 32 * g + 32, u, 4 * u + g] = fc3_w.reshape(32)
    b2r = np.tile(fc2_b.reshape(32, 1), (4, 1)).astype(f32)

    if "nc" not in _cache:
        _cache["nc"] = _build_program()
    nc = _cache["nc"]

    in_maps = []
    for c in range(NCORES):
        bs = slice(BPC * c, BPC * (c + 1))
        in_maps.append(
            {
                "crep": np.ascontiguousarray(crep[bs]),
                "a4": np.ascontiguousarray(a4[bs]),
                "w2d": w2d.astype(bf16),
                "w3d": w3d.reshape(D, 256).astype(bf16),
                "b2r": b2r,
            }
        )

    res = run_bass_kernel_spmd(nc, in_maps, core_ids=list(range(NCORES)))
    _last_results = res

    out = np.concatenate(
        [res.results[c]["out"][:, :N, :] for c in range(NCORES)], axis=0
    )
    b3 = float(fc3_b.reshape(-1)[0])
    if b3 != 0.0:
        out = out + b3
    return out.astype(f32)


# revision 17
# speedup vs baseline: 1.1781x; 1.0456x over previous
"""Trainium2 Bass kernel for nn_NNSDecoder (gnn_message_passing).

Reference computation (B=16, N=501, D=128, H=4):
    out[b,i,j] = fc3 . relu(fc2^T relu(feat @ fc1 + b1) + b2) + b3
    feat[b,i,j] = [cp_pre[b,i], cp_post[b,i], cd_pre[b,j], cd_post[b,j]]

Every compatibility term is linear in h_hat / h_nb rows, so folding the
head projections and fc1 gives per-batch N x 32 maps computed ON HOST
(O(N) work):
    A[b] = h_hat[b] @ G_A1 + h_nb[b] @ G_A2      (row/i term)
    C[b] = h_hat[b] @ G_C1 + h_nb[b] @ G_C2      (col/j term)
    out[b,i,j] = w3 . relu(W2^T relu(A[b,i] + C[b,j] + b1) + b2) + b3

Device inputs per batch: crep (bf16 [128, 504] = C^T + b1 stacked 4x)
and a4 (f32 [128, 126] = A in column-per-4-row-tile layout).

Device pipeline per batch (i-tiles of 4 rows, processed in pairs):
  - X(t) = relu(crep + a4[:,t])  on DVE (tensor_scalar, 2x mode)
  - fc2: ONE 1008-wide block-diag bf16 matmul per pair -> 2-bank pz2
  - Y = relu(pz2 + b2) -> bf16, one 1008-wide op (Scalar/DVE split)
  - fc3: ONE 1008-wide matmul per pair with one of 8 row-offset weight
    variants, accumulating 32 pairs (256 output rows, two 504-col
    halves per partition) into a dense 2-bank po2; po2 is copied once
    per 32 pairs and DMA'd out with an affine row remap.

Sharding: batch dim 16 -> 8 cores x 2 batches (data parallel).
"""

import numpy as np

B, N, D, H = 16, 501, 128, 4
NCORES = 8
BPC = B // NCORES  # batches per core
NP = 504  # padded N
NT = NP // 4  # 126 i-tiles of 4 rows
NPAIR = NT // 2  # 63 pairs per batch
PB = 512
NOUT = 512  # padded output rows per batch
PPPO = 32  # pairs per dense po2 (256 rows)

NTRI = NT // 3  # 42 tri-groups of 3 tiles per batch
# Y-tri engine split: of every 7 tri-groups, which go to Scalar vs DVE
Y_SPLIT = ("S", "S", "S", "D", "S", "S", "S")

_cache = {}


def _build_program():
    import concourse.bacc as bacc
    import concourse.mybir as mybir
    from concourse.tile import TileContext
    from concourse.bass_types import AP

    F32 = mybir.dt.float32
    BF16 = mybir.dt.bfloat16
    nc = bacc.Bacc("TRN2", target_bir_lowering=False, debug=False, num_devices=1)

    crep_d = nc.dram_tensor("crep", [BPC, D, NP], BF16, kind="ExternalInput")
    a4_d = nc.dram_tensor("a4", [BPC, D, NT], F32, kind="ExternalInput")
    w2_d = nc.dram_tensor("w2d", [D, D], BF16, kind="ExternalInput")
    w3_d = nc.dram_tensor("w3d", [D, 8 * 32], BF16, kind="ExternalInput")
    b2_d = nc.dram_tensor("b2r", [D, 1], F32, kind="ExternalInput")
    out = nc.dram_tensor("out", [BPC, NOUT, N], F32, kind="ExternalOutput")

    add = mybir.AluOpType.add
    amax = mybir.AluOpType.max
    Relu = mybir.ActivationFunctionType.Relu

    with TileContext(nc) as tc:
        with (
            tc.tile_pool(name="const", bufs=1) as cpool,
            tc.tile_pool(name="batch", bufs=2) as bpool,
            tc.tile_pool(name="x", bufs=12) as xpool,
            tc.tile_pool(name="y", bufs=6) as ypool,
            tc.tile_pool(name="o", bufs=2) as opool,
            tc.tile_pool(name="pz", bufs=2, space="PSUM") as pzpool,
            tc.tile_pool(name="po", bufs=1, space="PSUM") as popool,
        ):
            # ---- batch-0 critical inputs first; the rest deferred ----
            w2t = cpool.tile([D, D], BF16)
            nc.sync.dma_start(w2t[:], w2_d.ap()[:, :])
            b2t = cpool.tile([D, 1], F32)
            nc.gpsimd.dma_start(b2t[:], b2_d.ap()[:, :])

            batch_in = []
            for b in range(BPC):
                crep = bpool.tile([D, NP], BF16, tag="crep", name=f"crep{b}")
                a4 = bpool.tile([D, NT], F32, tag="a4", name=f"a4_{b}")
                batch_in.append((crep, a4))

            def issue_batch_dmas(b, engs, a4_eng):
                crep, a4 = batch_in[b]
                for c, e in enumerate(engs):
                    e.dma_start(
                        crep[32 * c : 32 * (c + 1), :],
                        crep_d.ap()[b, 32 * c : 32 * (c + 1), :],
                    )
                a4_eng.dma_start(a4[:], a4_d.ap()[b, :, :])

            issue_batch_dmas(0, [nc.sync, nc.scalar, nc.gpsimd, nc.sync],
                             nc.scalar)
            w3t = cpool.tile([D, 8 * 32], BF16)
            nc.gpsimd.dma_start(w3t[:], w3_d.ap()[:, :])

            for b in range(BPC):
                crep, a4 = batch_in[b]

                xq = {}

                def emit_x(t):
                    # tri tile [D, 1512]: X for tiles 3m..3m+2
                    m, s = divmod(t, 3)
                    if s == 0:
                        xq[m] = xpool.tile(
                            [D, 3 * NP], BF16, name=f"x{b}_{m}", tag="x"
                        )
                    x = xq[m]
                    nc.vector.tensor_scalar(
                        out=x[:, s * NP : s * NP + NP],
                        in0=crep[:],
                        scalar1=a4[:, t : t + 1],
                        scalar2=0.0,
                        op0=add,
                        op1=amax,
                    )

                po2_box = [None]

                def flush_po2(po2, pidx, last):
                    # dense po2: partition P = 4*sp + g, col 512q + j
                    #   -> out row 256*pidx + 8*sp + 4q + g
                    ob = opool.tile([D, 2 * PB], F32)
                    nc.scalar.copy(ob[:], po2[:])
                    base = ob[:, :]
                    pitch = base.ap[0][0]
                    engs = (
                        [nc.sync, nc.scalar, nc.gpsimd]
                        if last
                        else [nc.sync, nc.gpsimd]
                    )
                    k = 0
                    for g in range(4):
                        for q in range(2):
                            src = AP(
                                base.tensor,
                                base.offset + g * pitch + q * PB,
                                [
                                    [4 * pitch, PPPO],  # sp
                                    [1, N],  # j
                                ],
                            )
                            seg = out.ap()[b, :, :]
                            dst = AP(
                                seg.tensor,
                                seg.offset + (256 * pidx + 4 * q + g) * N,
                                [
                                    [8 * N, PPPO],  # sp
                                    [1, N],  # j
                                ],
                            )
                            engs[k % len(engs)].dma_start(dst, src)
                            k += 1

                def do_fc3(m, y3):
                    for s in range(3):
                        t = 3 * m + s
                        p, q = divmod(t, 2)
                        sp = p % PPPO
                        cg, u = divmod(sp, 8)
                        if sp == 0 and q == 0:
                            po2_box[0] = popool.tile(
                                [D, 2 * PB], F32,
                                name=f"po2_{b}_{p // PPPO}", tag="po",
                            )
                        po2 = po2_box[0]
                        nc.tensor.matmul(
                            po2[32 * cg : 32 * cg + 32, q * PB : q * PB + NP],
                            w3t[:, 32 * u : 32 * u + 32],
                            y3[:, s * PB : s * PB + NP],
                            start=(u == 0),
                            stop=(u == 7 or p == NPAIR - 1),
                            tile_position=(0, 32 * cg),
                            skip_group_check=True,
                        )
                        if q == 1 and (sp == PPPO - 1 or p == NPAIR - 1):
                            flush_po2(
                                po2, p // PPPO,
                                last=(b == BPC - 1 and p == NPAIR - 1),
                            )

                for t in range(6):
                    emit_x(t)
                pend = []
                for m in range(NTRI):
                    # fc2: three 504-wide matmuls into 3-bank pz3
                    pz3 = pzpool.tile([D, 3 * PB], F32, tag="pz")
                    xm = xq.pop(m)
                    for s in range(3):
                        nc.tensor.matmul(
                            pz3[:, s * PB : s * PB + NP],
                            w2t[:],
                            xm[:, s * NP : s * NP + NP],
                            start=True,
                            stop=True,
                        )
                    # Y = relu(pz3 + b2) -> bf16, one 1512-elem strided op
                    y3 = ypool.tile([D, 3 * PB], BF16, tag="y3")
                    y3v = y3.rearrange("p (q n) -> p q n", q=3)[:, :, 0:NP]
                    pz3v = pz3.rearrange("p (q n) -> p q n", q=3)[:, :, 0:NP]
                    if Y_SPLIT[m % len(Y_SPLIT)] == "D":
                        nc.vector.tensor_scalar(
                            out=y3v,
                            in0=pz3v,
                            scalar1=b2t[:, 0:1],
                            scalar2=0.0,
                            op0=add,
                            op1=amax,
                        )
                    else:
                        nc.scalar.activation(y3v, pz3v, Relu, bias=b2t[:, 0:1])
                    # X lookahead: tri m+2
                    for t in range(3 * m + 6, 3 * m + 9):
                        if t < NT:
                            emit_x(t)
                    if b == 0 and m == 1:
                        # batch-1 inputs: issue once batch-0 is rolling
                        issue_batch_dmas(
                            1, [nc.sync, nc.gpsimd, nc.sync, nc.gpsimd],
                            nc.gpsimd,
                        )
                    pend.append((m, y3))
                    if len(pend) > 2:
                        do_fc3(*pend.pop(0))
                for item in pend:
                    do_fc3(*item)

    nc.compile()
    return nc


def _host_prep(h_hat, pos_pickup, pos_delivery, solution, Wq1, Wk1, Wq2, Wk2,
               fc1_w, fc1_b):
    """Host: per-batch A, C maps (O(N*D) work), then crep/a4 layouts."""
    import ml_dtypes

    f32 = np.float32
    bf16 = ml_dtypes.bfloat16
    h_hat = np.asarray(h_hat, f32)
    pp = np.asarray(pos_pickup).astype(np.int64)
    pd = np.asarray(pos_delivery).astype(np.int64)
    sol = np.asarray(solution).astype(np.int64)
    Wq1 = np.asarray(Wq1, f32)
    Wk1 = np.asarray(Wk1, f32)
    Wq2 = np.asarray(Wq2, f32)
    Wk2 = np.asarray(Wk2, f32)
    fc1_w = np.asarray(fc1_w, f32)
    fc1_b = np.asarray(fc1_b, f32)

    crep = np.zeros((B, D, NP), bf16)
    a4 = np.zeros((B, D, NT), f32)

    for b in range(B):
        hb = h_hat[b]
        hnb = hb[sol[b]]
        p = hb[pp[b]]
        dv = hb[pd[b]]
        U1p = np.stack([Wk1[h] @ (Wq1[h].T @ p) for h in range(H)], axis=1)
        U2p = np.stack([Wk2[h] @ (Wq2[h].T @ p) for h in range(H)], axis=1)
        U1d = np.stack([Wk1[h] @ (Wq1[h].T @ dv) for h in range(H)], axis=1)
        U2d = np.stack([Wk2[h] @ (Wq2[h].T @ dv) for h in range(H)], axis=1)
        g1a = (U1p @ fc1_w[0:4]).astype(bf16).astype(f32)
        g2a = (U2p @ fc1_w[4:8]).astype(bf16).astype(f32)
        g1c = (U1d @ fc1_w[8:12]).astype(bf16).astype(f32)
        g2c = (U2d @ fc1_w[12:16]).astype(bf16).astype(f32)
        hbq = hb.astype(bf16).astype(f32)
        hnq = hnb.astype(bf16).astype(f32)
        A = hbq @ g1a + hnq @ g2a  # (N, 32)
        C = hbq @ g1c + hnq @ g2c  # (N, 32)
        CB = (C + fc1_b).astype(bf16)  # (N, 32)
        ct = np.tile(CB.T, (4, 1))  # (128, N)
        crep[b, :, :N] = ct
        crep[b, :, N:] = np.tile(fc1_b.reshape(32, 1).astype(bf16), (4, 1))
        Ap = np.zeros((NP, 32), f32)
        Ap[:N] = A
        a4[b] = Ap.reshape(NT, 4, 32).transpose(1, 2, 0).reshape(D, NT)
    return crep, a4


_last_results = None


def kernel(
    h_hat,
    pos_pickup,
    pos_delivery,
    solution,
    Wq1,
    Wk1,
    Wq2,
    Wk2,
    fc1_w,
    fc1_b,
    fc2_w,
    fc2_b,
    fc3_w,
    fc3_b,
):
    global _last_results
    import ml_dtypes
    from concourse.bass_utils import run_bass_kernel_spmd

    f32 = np.float32
    bf16 = ml_dtypes.bfloat16
    fc2_w = np.asarray(fc2_w, f32)
    fc2_b = np.asarray(fc2_b, f32)
    fc3_w = np.asarray(fc3_w, f32)
    fc3_b = np.asarray(fc3_b, f32)

    crep, a4 = _host_prep(
        h_hat, pos_pickup, pos_delivery, solution, Wq1, Wk1, Wq2, Wk2,
        np.asarray(fc1_w, f32), np.asarray(fc1_b, f32),
    )

    # block-diagonal packed fc2; 8 row-offset variants of fc3
    w2d = np.zeros((D, D), f32)
    for r in range(4):
        w2d[32 * r : 32 * r + 32, 32 * r : 32 * r + 32] = fc2_w
    w3d = np.zeros((D, 8, 32), f32)
    for u in range(8):
        for g in range(4):
            w3d[32 * g : 32 * g + 32, u, 4 * u + g] = fc3_w.reshape(32)
    b2r = np.tile(fc2_b.reshape(32, 1), (4, 1)).astype(f32)

    if "nc" not in _cache:
        _cache["nc"] = _build_program()
    nc = _cache["nc"]

    in_maps = []
    for c in range(NCORES):
        bs = slice(BPC * c, BPC * (c + 1))
        in_maps.append(
            {
                "crep": np.ascontiguousarray(crep[bs]),
                "a4": np.ascontiguousarray(a4[bs]),
                "w2d": w2d.astype(bf16),
                "w3d": w3d.reshape(D, 256).astype(bf16),
                "b2r": b2r,
            }
        )

    res = run_bass_kernel_spmd(nc, in_maps, core_ids=list(range(NCORES)))
    _last_results = res

    out = np.concatenate(
        [res.results[c]["out"][:, :N, :] for c in range(NCORES)], axis=0
    )
    b3 = float(fc3_b.reshape(-1)[0])
    if b3 != 0.0:
        out = out + b3
    return out.astype(f32)
